# revision 34
# baseline (speedup 1.0000x reference)
"""Distributed Trainium2 Bass kernel for multi-head attention.

Problem: x[2,2048,2048] @ qkv_w[2048,6144] -> rope(q,k) -> softmax(qk^T/sqrt(d)) @ v
         -> concat heads -> @ out_w[2048,2048].

Sharding (8 cores): core i handles batch b = i//4 and head group g = i%4
(heads 4g..4g+3).  All device tensors are pre-cast to bf16 on the host
(input DMA halves; no on-device cast traffic).  Each core:
  1. qT,kT = (Wqk_g^T x_b^T) with rope applied         [8 x [128, 2048]]
  2. v     = x_b @ Wv_g  (natural layout)              [16 x [128, 512]]
  3. attention chunked over q (4 x 512), ik-outer / head-inner:
     S^T = k q^T, P = exp(S^T/sqrt(d)), out^T = v^T P (PSUM-accumulated).
     Softmax denominators ("qsum"): P tiles pre-summed in quads on
     DVE/GpSimd, then one M=32 ones-matmul per quad per head, column-
     tiled to rows {0,32,64,96} of ONE psum bank pre-zeroed by DVE
     memset so all denominator matmuls run start=False (4x fewer PE
     cycles than per-tile ones-matmuls).  NOTE: custom DVE ops
     (reciprocal_approx_fast) mis-address PSUM partition offsets on HW
     -> denominator rows staged through ACT copies to partition-0 tiles.
  4. AllGather (bf16) attnT shards within the 4-core batch group, one AG
     per q-chunk, all issued before any output projection so AG latency
     hides behind remaining attention chunks.
  5. out[:, 512g:512(g+1)] = attnT_full^T @ out_w[:, 512g:512(g+1)],
     all four chunks after attention; own-group shard read from SBUF.
Host: slices/transposes/casts inputs per core, concatenates output columns.
"""

import numpy as np
import ml_dtypes

from concourse import bacc, mybir, tile
from concourse.bass_utils import run_bass_kernel_spmd

B, N, HID = 2, 2048, 2048
H, D = 16, 128
G = 4              # head groups (tensor parallel within a batch group)
HG = H // G        # heads per group
QK_COLS = HG * D   # 512
NT = N // 128      # 16 token tiles
KT = HID // 128    # 16 hidden tiles
TC = 512           # free-dim chunk (psum bank limit for f32)
NTC = N // TC      # 4
OC = HID // G      # 512 output columns per core

F32 = mybir.dt.float32
BF16 = mybir.dt.bfloat16
SCALE = float(1.0 / np.sqrt(D))
SWAP_MASK = [p ^ 1 for p in range(32)]  # adjacent-pair swap, uniform per 32-lane group
REPLICA_GROUPS = [[0, 1, 2, 3], [4, 5, 6, 7]]

_NC = None
LAST_RESULT = None


def _build(denom="qsum", rope_direct=True, cc_dt=BF16, debug_taps=False):
    nc = bacc.Bacc("TRN2", target_bir_lowering=False, debug=False, num_devices=8)

    xT = nc.dram_tensor("xT", [HID, N], BF16, kind="ExternalInput")
    wqk = nc.dram_tensor("wqk", [HID, 2 * QK_COLS], BF16, kind="ExternalInput")
    wv = nc.dram_tensor("wv", [HID, QK_COLS], BF16, kind="ExternalInput")
    wo = nc.dram_tensor("wo", [HID, OC], cc_dt, kind="ExternalInput")
    cosT = nc.dram_tensor("cosT", [D, N], BF16, kind="ExternalInput")
    sinT = nc.dram_tensor("sinT", [D, N], BF16, kind="ExternalInput")
    out = nc.dram_tensor("out", [N, OC], F32, kind="ExternalOutput")
    if debug_taps:
        dbg_q = nc.dram_tensor("dbg_q", [128, N], BF16, kind="ExternalOutput")
        dbg_k = nc.dram_tensor("dbg_k", [128, N], BF16, kind="ExternalOutput")
        dbg_v = nc.dram_tensor("dbg_v", [128, QK_COLS], BF16, kind="ExternalOutput")
        dbg_d = nc.dram_tensor("dbg_d", [NTC * HG, TC], F32, kind="ExternalOutput")
        dbg_a = nc.dram_tensor("dbg_a", [128, TC], BF16, kind="ExternalOutput")
        dbg_cc = nc.dram_tensor("dbg_cc", [G * HG * 128, TC], cc_dt,
                                kind="ExternalOutput")

    with tile.TileContext(nc) as tc:
        with (
            tc.tile_pool(name="dram", bufs=1, space="DRAM") as dram,
            tc.tile_pool(name="persist", bufs=1) as persist,
            tc.tile_pool(name="s5at", bufs=2) as s5at,
        ):
            def atb_load(jq, eng_even=None, eng_odd=None):
                tiles = []
                for k3 in range(KT):
                    t = s5at.tile([128, TC], BF16, name=f"at{k3}",
                                  tag=f"at{k3 % 8}", bufs=3)
                    eng = eng_even if k3 % 2 == 0 else eng_odd
                    eng.dma_start(t[:], cc_out[jq][k3 * 128:(k3 + 1) * 128, :])
                    tiles.append(t)
                return tiles

            qkT = [persist.tile([128, N], BF16, name=f"qkT{m}", tag=f"qkT{m}")
                   for m in range(2 * HG)]
            v_sb = [persist.tile([128, QK_COLS], BF16, name=f"v{t}", tag=f"v{t}")
                    for t in range(NT)]
            wo_sb = [persist.tile([128, OC], BF16, name=f"wo{k}", tag=f"wo{k}")
                     for k in range(KT)]
            # own head-group attention output, kept in SBUF through outproj
            asb = [[persist.tile([128, TC], BF16, name=f"asb{j}_{h}", tag=f"asb{j}_{h}")
                    for h in range(HG)] for j in range(NTC)]
            ones_sb = persist.tile([128, 32], BF16, name="ones", tag="ones")
            nc.vector.memset(ones_sb[:], 1.0)

            cc_in = [dram.tile([HG * 128, TC], cc_dt, name=f"cc_in{j}", tag=f"cc_in{j}")
                     for j in range(NTC)]
            cc_out = [dram.tile([G * HG * 128, TC], cc_dt, name=f"cc_out{j}", tag=f"cc_out{j}")
                      for j in range(NTC)]

            # ---- stage 1+2: q,k (transposed, roped) and v (natural) ----
            with (
                tc.tile_pool(name="s1w", bufs=1) as s1w,
                tc.tile_pool(name="s1x", bufs=1) as s1x,
                tc.tile_pool(name="s1t", bufs=2) as s1t,
                tc.tile_pool(name="psqk", bufs=6, space="PSUM") as psqk,
                tc.tile_pool(name="psv", bufs=2, space="PSUM") as psv,
            ):
                cos_sb = s1w.tile([D, N], BF16, name="cos", tag="cos")
                sin_sb = s1w.tile([D, N], BF16, name="sin", tag="sin")
                nc.sync.dma_start(cos_sb[:], cosT[:])
                nc.sync.dma_start(sin_sb[:], sinT[:])
                wqk_sb = [s1w.tile([128, 2 * QK_COLS], BF16, name=f"wqk{k}", tag=f"wqk{k}")
                          for k in range(KT)]
                wv_sb = [s1w.tile([128, QK_COLS], BF16, name=f"wv{k}", tag=f"wv{k}")
                        for k in range(KT)]
                for k in range(KT):
                    nc.sync.dma_start(wqk_sb[k][:], wqk[k * 128:(k + 1) * 128, :])
                    nc.scalar.dma_start(wv_sb[k][:], wv[k * 128:(k + 1) * 128, :])

                def load_xt(tcn):
                    tsl = slice(tcn * TC, (tcn + 1) * TC)
                    xt = [s1x.tile([128, TC], BF16, name=f"xt{k}", tag=f"xt{k}", bufs=2)
                          for k in range(KT)]
                    for k in range(KT):
                        nc.gpsimd.dma_start(xt[k][:], xT[k * 128:(k + 1) * 128, tsl])
                    return xt

                xt = load_xt(0)
                for tcn in range(NTC):
                    scope = nc.named_scope(f"qkv{tcn}")
                    scope.__enter__()
                    tsl = slice(tcn * TC, (tcn + 1) * TC)
                    for half in range(2):
                        psums = [psqk.tile([128, TC], F32, name="psqk", tag="psqk")
                                 for _ in range(4)]
                        for k in range(KT):
                            for mi in range(4):
                                m = half * 4 + mi
                                nc.tensor.matmul(
                                    psums[mi][:],
                                    wqk_sb[k][:, m * 128:(m + 1) * 128],
                                    xt[k][:],
                                    start=(k == 0),
                                    stop=(k == KT - 1),
                                )
                        for mi in range(4):
                            m = half * 4 + mi
                            if rope_direct:
                                src = psums[mi]
                            else:
                                src = s1t.tile([128, TC], BF16, tag="qsb")
                                nc.scalar.activation(
                                    src[:], psums[mi][:],
                                    mybir.ActivationFunctionType.Copy,
                                )
                            shuf = s1t.tile([128, TC], F32, tag="shuf")
                            nc.vector.stream_shuffle(shuf[:], src[:], SWAP_MASK)
                            t1 = s1t.tile([128, TC], F32, tag="t1")
                            nc.vector.tensor_tensor(
                                t1[:], src[:], cos_sb[:, tsl], mybir.AluOpType.mult
                            )
                            t2 = s1t.tile([128, TC], F32, tag="t2")
                            nc.vector.tensor_tensor(
                                t2[:], shuf[:], sin_sb[:, tsl], mybir.AluOpType.mult
                            )
                            nc.vector.tensor_tensor(
                                qkT[m][:, tsl], t1[:], t2[:], mybir.AluOpType.add
                            )
                    xt_next = load_xt(tcn + 1) if tcn + 1 < NTC else None
                    for mtl in range(4):
                        mt = tcn * 4 + mtl
                        pv = psv.tile([128, QK_COLS], F32, name="psv", tag="psv")
                        for k in range(KT):
                            nc.tensor.matmul(
                                pv[:],
                                xt[k][:, mtl * 128:(mtl + 1) * 128],
                                wv_sb[k][:],
                                start=(k == 0),
                                stop=(k == KT - 1),
                            )
                        nc.scalar.activation(
                            v_sb[mt][:], pv[:], mybir.ActivationFunctionType.Copy
                        )
                    xt = xt_next
                    scope.__exit__(None, None, None)

            for k in range(KT):
                nc.sync.dma_start(wo_sb[k][:], wo[k * 128:(k + 1) * 128, :])

            # ---- stages 3+4: attention chunks, AG per chunk ----
            with (
                tc.tile_pool(name="s3p", bufs=8) as s3p,
                tc.tile_pool(name="s3d", bufs=2) as s3d,
                tc.tile_pool(name="pso", bufs=1, space="PSUM") as pso,
                tc.tile_pool(name="psd", bufs=1, space="PSUM") as psdp,
                tc.tile_pool(name="pss", bufs=4 if denom == "headouter" else 3,
                             space="PSUM") as pss,
            ):
                def normalize_store(jq, h, po_t, den_ap):
                    # den_ap sits at psum partition 32h; custom DVE ops
                    # mis-address partition offsets, so stage through a
                    # base-partition-0 SBUF tile with a standard ACT copy.
                    dcp = s3d.tile([1, TC], F32, name="dcp", tag="dcp")
                    nc.scalar.activation(
                        dcp[:], den_ap, mybir.ActivationFunctionType.Copy
                    )
                    dr = s3d.tile([1, TC], F32, name="dr", tag="dr")
                    nc.vector.reciprocal_approx_fast(dr[:], dcp[:])
                    drb = s3d.tile([128, TC], F32, name="drb", tag="drb")
                    nc.gpsimd.partition_broadcast(drb[:], dr[:])
                    nc.vector.tensor_tensor(
                        asb[jq][h][:], po_t[:], drb[:], mybir.AluOpType.mult
                    )
                    nc.sync.dma_start(
                        cc_in[jq][h * 128:(h + 1) * 128, :], asb[jq][h][:]
                    )
                    if debug_taps:
                        nc.scalar.dma_start(
                            dbg_d[jq * HG + h:jq * HG + h + 1, :], dr[:]
                        )

                def attn_chunk_ikouter(jq):
                    qsl = slice(jq * TC, (jq + 1) * TC)
                    psd = psdp.tile([128, TC], F32, name="psd", tag="psd")
                    nc.vector.memset(psd[:], 0.0)
                    po = [pso.tile([128, TC], F32, name=f"po{h}", tag=f"po{h}")
                          for h in range(HG)]
                    pair_a = [None] * HG
                    pair_b = [None] * HG
                    p_hist = []
                    pending = []
                    for ik in range(NT):
                        ksl = slice(ik * 128, (ik + 1) * 128)
                        if denom == "qsum" and ik % 4 == 1 and pending:
                            # flush previous quad's denominator matmuls --
                            # one quad of slack lets the DVE adds complete
                            # off the PE critical path
                            for h, qs_t in pending:
                                dmm(h, qs_t, False)
                            pending = []
                        ps = [pss.tile([128, TC], F32, name="pss", tag="pss")
                              for _ in range(HG)]
                        for h in range(HG):
                            nc.tensor.matmul(
                                ps[h][:], qkT[HG + h][:, ksl], qkT[h][:, qsl],
                                start=True, stop=True,
                            )
                        p = [s3p.tile([128, TC], BF16, name="p", tag=f"p{h}",
                                      bufs=4)
                             for h in range(HG)]
                        for h in range(HG):
                            nc.scalar.activation(
                                p[h][:], ps[h][:],
                                mybir.ActivationFunctionType.Exp, scale=SCALE,
                            )
                        def dmm(h, rhs, stop):
                            # denominator: M=32 col-tile at rows 32h of psd
                            # (32 identical rows of ones), accumulating onto
                            # the memset-zeroed bank with start=False.
                            nc.tensor.matmul(
                                psd[32 * h:32 * h + 32, :],
                                ones_sb[:, 0:32],
                                rhs[:],
                                start=False,
                                stop=stop,
                                skip_group_check=True,
                                tile_position=(0, 32 * h),
                            )
                        for h in range(HG):
                            nc.tensor.matmul(
                                po[h][:],
                                v_sb[ik][:, h * 128:(h + 1) * 128],
                                p[h][:],
                                start=(ik == 0),
                                stop=(ik == NT - 1),
                            )
                            if denom == "spread":
                                dmm(h, p[h], ik == NT - 1)
                        if denom == "pack":
                            for h in range(HG):
                                dmm(h, p[h], ik == NT - 1)
                        elif denom == "qsum":
                            # pre-sum quads of p on DVE/GpSimd; one ones-
                            # matmul per quad instead of per ik tile
                            if ik % 4 == 1:
                                for h in range(HG):
                                    pair_a[h] = s3p.tile(
                                        [128, TC], BF16, name="pa",
                                        tag=f"pa{h}", bufs=2
                                    )
                                    nc.vector.tensor_tensor(
                                        pair_a[h][:], p_hist[-1][h][:], p[h][:],
                                        mybir.AluOpType.add,
                                    )
                            elif ik % 4 == 3:
                                for h in range(HG):
                                    pair_b[h] = s3p.tile(
                                        [128, TC], BF16, name="pb",
                                        tag=f"pb{h}", bufs=2
                                    )
                                    nc.vector.tensor_tensor(
                                        pair_b[h][:], p_hist[-1][h][:], p[h][:],
                                        mybir.AluOpType.add,
                                    )
                                for h in range(HG):
                                    qs = s3p.tile(
                                        [128, TC], BF16, name="qs",
                                        tag=f"qs{h}", bufs=2
                                    )
                                    nc.vector.tensor_tensor(
                                        qs[:], pair_a[h][:], pair_b[h][:],
                                        mybir.AluOpType.add,
                                    )
                                    pending.append((h, qs))
                        p_hist.append(p)
                        if len(p_hist) > 2:
                            p_hist.pop(0)
                    for h, qs_t in pending:
                        dmm(h, qs_t, True)
                    for h in range(HG):
                        normalize_store(jq, h, po[h], psd[32 * h:32 * h + 1, :])

                def attn_chunk_headouter(jq):
                    qsl = slice(jq * TC, (jq + 1) * TC)
                    for h in range(HG):
                        po_t = pso.tile([128, TC], F32, name="po0", tag="po0")
                        pdt = psdp.tile([128, TC], F32, name="psd", tag="psd")
                        pd = pdt[0:1, :]
                        for ik in range(NT):
                            ksl = slice(ik * 128, (ik + 1) * 128)
                            ps = pss.tile([128, TC], F32, name="pss", tag="pss")
                            nc.tensor.matmul(
                                ps[:], qkT[HG + h][:, ksl], qkT[h][:, qsl],
                                start=True, stop=True,
                            )
                            p = s3p.tile([128, TC], BF16, name="p", tag="p")
                            nc.scalar.activation(
                                p[:], ps[:],
                                mybir.ActivationFunctionType.Exp, scale=SCALE,
                            )
                            nc.tensor.matmul(
                                po_t[:], v_sb[ik][:, h * 128:(h + 1) * 128], p[:],
                                start=(ik == 0), stop=(ik == NT - 1),
                            )
                            nc.tensor.matmul(
                                pd, ones_sb[:, 0:1], p[:],
                                start=(ik == 0), stop=(ik == NT - 1),
                            )
                        normalize_store(jq, h, po_t, pd)

                atb0 = None
                for jq in range(NTC):
                    scope = nc.named_scope(f"attn{jq}")
                    scope.__enter__()
                    if denom == "headouter":
                        attn_chunk_headouter(jq)
                    else:
                        attn_chunk_ikouter(jq)
                    nc.gpsimd.collective_compute(
                        "AllGather",
                        mybir.AluOpType.bypass,
                        replica_groups=REPLICA_GROUPS,
                        ins=[cc_in[jq].opt()],
                        outs=[cc_out[jq].opt()],
                    )
                    if jq == 1:
                        # prefetch chunk-0 atb during attn2 (AG0 done by then)
                        atb0 = atb_load(0, nc.gpsimd, nc.gpsimd)
                    scope.__exit__(None, None, None)
                if debug_taps:
                    nc.scalar.dma_start(dbg_q[:], qkT[0][:])
                    nc.scalar.dma_start(dbg_k[:], qkT[HG][:])
                    nc.scalar.dma_start(dbg_v[:], v_sb[0][:])
                    nc.scalar.dma_start(dbg_a[:], asb[0][0][:])
                    nc.scalar.dma_start(dbg_cc[:], cc_out[0][:])

            # ---- stage 5: output projection, all chunks ----
            with (
                tc.tile_pool(name="s5o", bufs=3) as s5o,
                tc.tile_pool(name="psf", bufs=2, space="PSUM") as psf,
            ):
                for jq in range(NTC):
                    scope = nc.named_scope(f"proj{jq}")
                    scope.__enter__()
                    atb = atb0 if jq == 0 else atb_load(jq, nc.sync, nc.scalar)
                    for mql in range(TC // 128):
                        mq = jq * (TC // 128) + mql
                        pf = psf.tile([128, OC], F32, name="psf", tag="psf")
                        for k3 in range(KT):
                            nc.tensor.matmul(
                                pf[:],
                                atb[k3][:, mql * 128:(mql + 1) * 128],
                                wo_sb[k3][:],
                                start=(k3 == 0),
                                stop=(k3 == KT - 1),
                            )
                        ob = s5o.tile([128, OC], F32, name="ob", tag="ob")
                        nc.scalar.activation(
                            ob[:], pf[:], mybir.ActivationFunctionType.Copy
                        )
                        nc.gpsimd.dma_start(out[mq * 128:(mq + 1) * 128, :], ob[:])
                    scope.__exit__(None, None, None)

    nc.compile()
    return nc


def _get_nc():
    global _NC
    if _NC is None:
        _NC = _build()
    return _NC


def _bf16(a):
    return np.ascontiguousarray(np.asarray(a).astype(ml_dtypes.bfloat16))


def _prep_in_maps(x, rope, qkv_w, out_w):
    x = np.asarray(x, np.float32)
    rope = np.asarray(rope, np.float32)
    qkv_w = np.asarray(qkv_w, np.float32)
    out_w = np.asarray(out_w, np.float32)

    freqs = rope[:, 0, :]  # [N, D]
    cosT = np.repeat(freqs[:, 0::2], 2, axis=1).T  # [D, N]
    sinT = np.repeat(freqs[:, 1::2], 2, axis=1).T.copy()
    sinT[0::2, :] *= -1.0  # rope sign folded in: rot[2i] = -q[2i+1]

    qkv3 = qkv_w.reshape(HID, 3, H, D)
    xTs = [_bf16(x[b].T) for b in range(B)]
    cosT_b, sinT_b = _bf16(cosT), _bf16(sinT)
    in_maps = []
    for core in range(8):
        b, g = core // G, core % G
        hs = slice(g * HG, (g + 1) * HG)
        wq = qkv3[:, 0, hs, :].reshape(HID, QK_COLS)
        wk = qkv3[:, 1, hs, :].reshape(HID, QK_COLS)
        in_maps.append(
            dict(
                xT=xTs[b],
                wqk=_bf16(np.concatenate([wq, wk], axis=1)),
                wv=_bf16(qkv3[:, 2, hs, :].reshape(HID, QK_COLS)),
                wo=_bf16(out_w[:, g * OC:(g + 1) * OC]),
                cosT=cosT_b,
                sinT=sinT_b,
            )
        )
    return in_maps


def kernel(x, rope, qkv_w, out_w):
    global LAST_RESULT
    nc = _get_nc()
    in_maps = _prep_in_maps(x, rope, qkv_w, out_w)
    res = run_bass_kernel_spmd(nc, in_maps, core_ids=list(range(8)))
    LAST_RESULT = res
    outs = [r["out"] for r in res.results]
    full = np.stack(
        [np.concatenate([outs[b * G + g] for g in range(G)], axis=1) for b in range(B)]
    )
    return full.astype(np.float32)


# revision 36
# speedup vs baseline: 1.0184x; 1.0184x over previous
"""Distributed Trainium2 Bass kernel for multi-head attention.

Problem: x[2,2048,2048] @ qkv_w[2048,6144] -> rope(q,k) -> softmax(qk^T/sqrt(d)) @ v
         -> concat heads -> @ out_w[2048,2048].

Sharding (8 cores): core i handles batch b = i//4 and head group g = i%4
(heads 4g..4g+3).  All device tensors are pre-cast to bf16 on the host
(input DMA halves; no on-device cast traffic).  Each core:
  1. qT,kT = (Wqk_g^T x_b^T) with rope applied         [8 x [128, 2048]]
  2. v     = x_b @ Wv_g  (natural layout)              [16 x [128, 512]]
  3. attention chunked over q (4 x 512), ik-outer / head-inner:
     S^T = k q^T, P = exp(S^T/sqrt(d)), out^T = v^T P (PSUM-accumulated).
     Softmax denominators ("qsum"): P tiles pre-summed in quads on
     DVE/GpSimd, then one M=32 ones-matmul per quad per head, column-
     tiled to rows {0,32,64,96} of ONE psum bank pre-zeroed by DVE
     memset so all denominator matmuls run start=False (4x fewer PE
     cycles than per-tile ones-matmuls).  NOTE: custom DVE ops
     (reciprocal_approx_fast) mis-address PSUM partition offsets on HW
     -> denominator rows staged through ACT copies to partition-0 tiles.
  4. AllGather (bf16) attnT shards within the 4-core batch group, one AG
     per q-chunk, all issued before any output projection so AG latency
     hides behind remaining attention chunks.
  5. out[:, 512g:512(g+1)] = attnT_full^T @ out_w[:, 512g:512(g+1)],
     all four chunks after attention; own-group shard read from SBUF.
Host: slices/transposes/casts inputs per core, concatenates output columns.
"""

import numpy as np
import ml_dtypes

from concourse import bacc, mybir, tile
from concourse.bass_utils import run_bass_kernel_spmd

B, N, HID = 2, 2048, 2048
H, D = 16, 128
G = 4              # head groups (tensor parallel within a batch group)
HG = H // G        # heads per group
QK_COLS = HG * D   # 512
NT = N // 128      # 16 token tiles
KT = HID // 128    # 16 hidden tiles
TC = 512           # free-dim chunk (psum bank limit for f32)
NTC = N // TC      # 4
OC = HID // G      # 512 output columns per core

F32 = mybir.dt.float32
BF16 = mybir.dt.bfloat16
SCALE = float(1.0 / np.sqrt(D))
SWAP_MASK = [p ^ 1 for p in range(32)]  # adjacent-pair swap, uniform per 32-lane group
REPLICA_GROUPS = [[0, 1, 2, 3], [4, 5, 6, 7]]

_NC = None
LAST_RESULT = None


def _build(denom="qsum", rope_direct=True, cc_dt=BF16, debug_taps=False):
    nc = bacc.Bacc("TRN2", target_bir_lowering=False, debug=False, num_devices=8)

    xT = nc.dram_tensor("xT", [HID, N], BF16, kind="ExternalInput")
    wqk = nc.dram_tensor("wqk", [HID, 2 * QK_COLS], BF16, kind="ExternalInput")
    wv = nc.dram_tensor("wv", [HID, QK_COLS], BF16, kind="ExternalInput")
    wo = nc.dram_tensor("wo", [HID, OC], cc_dt, kind="ExternalInput")
    cosT = nc.dram_tensor("cosT", [D, N], BF16, kind="ExternalInput")
    sinT = nc.dram_tensor("sinT", [D, N], BF16, kind="ExternalInput")
    out = nc.dram_tensor("out", [N, OC], F32, kind="ExternalOutput")
    if debug_taps:
        dbg_q = nc.dram_tensor("dbg_q", [128, N], BF16, kind="ExternalOutput")
        dbg_k = nc.dram_tensor("dbg_k", [128, N], BF16, kind="ExternalOutput")
        dbg_v = nc.dram_tensor("dbg_v", [128, QK_COLS], BF16, kind="ExternalOutput")
        dbg_d = nc.dram_tensor("dbg_d", [NTC * HG, TC], F32, kind="ExternalOutput")
        dbg_a = nc.dram_tensor("dbg_a", [128, TC], BF16, kind="ExternalOutput")
        dbg_cc = nc.dram_tensor("dbg_cc", [G * HG * 128, TC], cc_dt,
                                kind="ExternalOutput")

    with tile.TileContext(nc) as tc:
        with (
            tc.tile_pool(name="dram", bufs=1, space="DRAM") as dram,
            tc.tile_pool(name="persist", bufs=1) as persist,
            tc.tile_pool(name="s5at", bufs=2) as s5at,
        ):
            def atb_load(jq, eng_even=None, eng_odd=None):
                tiles = []
                for k3 in range(KT):
                    t = s5at.tile([128, TC], BF16, name=f"at{k3}",
                                  tag=f"at{k3 % 8}", bufs=3)
                    eng = eng_even if k3 % 2 == 0 else eng_odd
                    eng.dma_start(t[:], cc_out[jq][k3 * 128:(k3 + 1) * 128, :])
                    tiles.append(t)
                return tiles

            qkT = [[persist.tile([128, TC], BF16, name=f"qkT{m}_{t}",
                                 tag=f"qkT{m}_{t}") for t in range(NTC)]
                   for m in range(2 * HG)]
            v_sb = [persist.tile([128, QK_COLS], BF16, name=f"v{t}", tag=f"v{t}")
                    for t in range(NT)]
            wo_sb = [persist.tile([128, OC], BF16, name=f"wo{k}", tag=f"wo{k}")
                     for k in range(KT)]
            # own head-group attention output, kept in SBUF through outproj
            asb = [[persist.tile([128, TC], BF16, name=f"asb{j}_{h}", tag=f"asb{j}_{h}")
                    for h in range(HG)] for j in range(NTC)]
            ones_sb = persist.tile([128, 32], BF16, name="ones", tag="ones")
            nc.vector.memset(ones_sb[:], 1.0)

            cc_in = [dram.tile([HG * 128, TC], cc_dt, name=f"cc_in{j}", tag=f"cc_in{j}")
                     for j in range(NTC)]
            cc_out = [dram.tile([G * HG * 128, TC], cc_dt, name=f"cc_out{j}", tag=f"cc_out{j}")
                      for j in range(NTC)]

            # ---- stage 1+2: q,k (transposed, roped) and v (natural) ----
            with (
                tc.tile_pool(name="s1w", bufs=1) as s1w,
                tc.tile_pool(name="s1x", bufs=1) as s1x,
                tc.tile_pool(name="s1t", bufs=2) as s1t,
                tc.tile_pool(name="psqk", bufs=6, space="PSUM") as psqk,
                tc.tile_pool(name="psv", bufs=2, space="PSUM") as psv,
            ):
                cos_sb = s1w.tile([D, N], BF16, name="cos", tag="cos")
                sin_sb = s1w.tile([D, N], BF16, name="sin", tag="sin")
                nc.sync.dma_start(cos_sb[:], cosT[:])
                nc.sync.dma_start(sin_sb[:], sinT[:])
                wqk_sb = [s1w.tile([128, 2 * QK_COLS], BF16, name=f"wqk{k}", tag=f"wqk{k}")
                          for k in range(KT)]
                wv_sb = [s1w.tile([128, QK_COLS], BF16, name=f"wv{k}", tag=f"wv{k}")
                        for k in range(KT)]
                for k in range(KT):
                    nc.sync.dma_start(wqk_sb[k][:], wqk[k * 128:(k + 1) * 128, :])
                    nc.scalar.dma_start(wv_sb[k][:], wv[k * 128:(k + 1) * 128, :])

                def load_xt(tcn):
                    tsl = slice(tcn * TC, (tcn + 1) * TC)
                    xt = [s1x.tile([128, TC], BF16, name=f"xt{k}", tag=f"xt{k}", bufs=2)
                          for k in range(KT)]
                    for k in range(KT):
                        nc.gpsimd.dma_start(xt[k][:], xT[k * 128:(k + 1) * 128, tsl])
                    return xt

                xt = load_xt(0)
                for tcn in range(NTC):
                    scope = nc.named_scope(f"qkv{tcn}")
                    scope.__enter__()
                    tsl = slice(tcn * TC, (tcn + 1) * TC)
                    for half in range(2):
                        psums = [psqk.tile([128, TC], F32, name="psqk", tag="psqk")
                                 for _ in range(4)]
                        for k in range(KT):
                            for mi in range(4):
                                m = half * 4 + mi
                                nc.tensor.matmul(
                                    psums[mi][:],
                                    wqk_sb[k][:, m * 128:(m + 1) * 128],
                                    xt[k][:],
                                    start=(k == 0),
                                    stop=(k == KT - 1),
                                )
                        for mi in range(4):
                            m = half * 4 + mi
                            if rope_direct:
                                src = psums[mi]
                            else:
                                src = s1t.tile([128, TC], BF16, tag="qsb")
                                nc.scalar.activation(
                                    src[:], psums[mi][:],
                                    mybir.ActivationFunctionType.Copy,
                                )
                            shuf = s1t.tile([128, TC], F32, tag="shuf")
                            nc.vector.stream_shuffle(shuf[:], src[:], SWAP_MASK)
                            t1 = s1t.tile([128, TC], F32, tag="t1")
                            nc.vector.tensor_tensor(
                                t1[:], src[:], cos_sb[:, tsl], mybir.AluOpType.mult
                            )
                            t2 = s1t.tile([128, TC], F32, tag="t2")
                            nc.vector.tensor_tensor(
                                t2[:], shuf[:], sin_sb[:, tsl], mybir.AluOpType.mult
                            )
                            nc.vector.tensor_tensor(
                                qkT[m][tcn][:], t1[:], t2[:], mybir.AluOpType.add
                            )
                    xt_next = load_xt(tcn + 1) if tcn + 1 < NTC else None
                    for mtl in range(4):
                        mt = tcn * 4 + mtl
                        pv = psv.tile([128, QK_COLS], F32, name="psv", tag="psv")
                        for k in range(KT):
                            nc.tensor.matmul(
                                pv[:],
                                xt[k][:, mtl * 128:(mtl + 1) * 128],
                                wv_sb[k][:],
                                start=(k == 0),
                                stop=(k == KT - 1),
                            )
                        nc.scalar.activation(
                            v_sb[mt][:], pv[:], mybir.ActivationFunctionType.Copy
                        )
                    xt = xt_next
                    scope.__exit__(None, None, None)

            for k in range(KT):
                nc.sync.dma_start(wo_sb[k][:], wo[k * 128:(k + 1) * 128, :])

            # ---- stages 3+4: attention chunks, AG per chunk ----
            with (
                tc.tile_pool(name="s3p", bufs=8) as s3p,
                tc.tile_pool(name="s3d", bufs=2) as s3d,
                tc.tile_pool(name="pso", bufs=1, space="PSUM") as pso,
                tc.tile_pool(name="psd", bufs=1, space="PSUM") as psdp,
                tc.tile_pool(name="pss", bufs=4 if denom == "headouter" else 3,
                             space="PSUM") as pss,
            ):
                def normalize_store(jq, h, po_t, den_ap):
                    # den_ap sits at psum partition 32h; custom DVE ops
                    # mis-address partition offsets, so stage through a
                    # base-partition-0 SBUF tile with a standard ACT copy.
                    dcp = s3d.tile([1, TC], F32, name="dcp", tag="dcp")
                    nc.scalar.activation(
                        dcp[:], den_ap, mybir.ActivationFunctionType.Copy
                    )
                    dr = s3d.tile([1, TC], F32, name="dr", tag="dr")
                    nc.vector.reciprocal_approx_fast(dr[:], dcp[:])
                    drb = s3d.tile([128, TC], F32, name="drb", tag="drb")
                    nc.gpsimd.partition_broadcast(drb[:], dr[:])
                    nc.vector.tensor_tensor(
                        asb[jq][h][:], po_t[:], drb[:], mybir.AluOpType.mult
                    )
                    nc.sync.dma_start(
                        cc_in[jq][h * 128:(h + 1) * 128, :], asb[jq][h][:]
                    )
                    if debug_taps:
                        nc.scalar.dma_start(
                            dbg_d[jq * HG + h:jq * HG + h + 1, :], dr[:]
                        )

                def attn_chunk_ikouter(jq):
                    qsl = slice(jq * TC, (jq + 1) * TC)
                    psd = psdp.tile([128, TC], F32, name="psd", tag="psd")
                    nc.vector.memset(psd[:], 0.0)
                    po = [pso.tile([128, TC], F32, name=f"po{h}", tag=f"po{h}")
                          for h in range(HG)]
                    pair_a = [None] * HG
                    pair_b = [None] * HG
                    p_hist = []
                    pending = []
                    for ik in range(NT):
                        ksl = slice(ik * 128, (ik + 1) * 128)
                        if denom == "qsum" and ik % 4 == 1 and pending:
                            # flush previous quad's denominator matmuls --
                            # one quad of slack lets the DVE adds complete
                            # off the PE critical path
                            for h, qs_t in pending:
                                dmm(h, qs_t, False)
                            pending = []
                        ps = [pss.tile([128, TC], F32, name="pss", tag="pss")
                              for _ in range(HG)]
                        kc, ko = ik // 4, (ik % 4) * 128
                        for h in range(HG):
                            nc.tensor.matmul(
                                ps[h][:], qkT[HG + h][kc][:, ko:ko + 128],
                                qkT[h][jq][:],
                                start=True, stop=True,
                            )
                        p = [s3p.tile([128, TC], BF16, name="p", tag=f"p{h}",
                                      bufs=4)
                             for h in range(HG)]
                        for h in range(HG):
                            nc.scalar.activation(
                                p[h][:], ps[h][:],
                                mybir.ActivationFunctionType.Exp, scale=SCALE,
                            )
                        def dmm(h, rhs, stop):
                            # denominator: M=32 col-tile at rows 32h of psd
                            # (32 identical rows of ones), accumulating onto
                            # the memset-zeroed bank with start=False.
                            nc.tensor.matmul(
                                psd[32 * h:32 * h + 32, :],
                                ones_sb[:, 0:32],
                                rhs[:],
                                start=False,
                                stop=stop,
                                skip_group_check=True,
                                tile_position=(0, 32 * h),
                            )
                        for h in range(HG):
                            nc.tensor.matmul(
                                po[h][:],
                                v_sb[ik][:, h * 128:(h + 1) * 128],
                                p[h][:],
                                start=(ik == 0),
                                stop=(ik == NT - 1),
                            )
                            if denom == "spread":
                                dmm(h, p[h], ik == NT - 1)
                        if denom == "pack":
                            for h in range(HG):
                                dmm(h, p[h], ik == NT - 1)
                        elif denom == "qsum":
                            # pre-sum quads of p on DVE/GpSimd; one ones-
                            # matmul per quad instead of per ik tile
                            if ik % 4 == 1:
                                for h in range(HG):
                                    pair_a[h] = s3p.tile(
                                        [128, TC], BF16, name="pa",
                                        tag=f"pa{h}", bufs=2
                                    )
                                    nc.vector.tensor_tensor(
                                        pair_a[h][:], p_hist[-1][h][:], p[h][:],
                                        mybir.AluOpType.add,
                                    )
                            elif ik % 4 == 3:
                                for h in range(HG):
                                    pair_b[h] = s3p.tile(
                                        [128, TC], BF16, name="pb",
                                        tag=f"pb{h}", bufs=2
                                    )
                                    nc.vector.tensor_tensor(
                                        pair_b[h][:], p_hist[-1][h][:], p[h][:],
                                        mybir.AluOpType.add,
                                    )
                                for h in range(HG):
                                    qs = s3p.tile(
                                        [128, TC], BF16, name="qs",
                                        tag=f"qs{h}", bufs=2
                                    )
                                    nc.vector.tensor_tensor(
                                        qs[:], pair_a[h][:], pair_b[h][:],
                                        mybir.AluOpType.add,
                                    )
                                    pending.append((h, qs))
                        p_hist.append(p)
                        if len(p_hist) > 2:
                            p_hist.pop(0)
                    for h, qs_t in pending:
                        dmm(h, qs_t, True)
                    for h in range(HG):
                        normalize_store(jq, h, po[h], psd[32 * h:32 * h + 1, :])

                def attn_chunk_headouter(jq):
                    qsl = slice(jq * TC, (jq + 1) * TC)
                    for h in range(HG):
                        po_t = pso.tile([128, TC], F32, name="po0", tag="po0")
                        pdt = psdp.tile([128, TC], F32, name="psd", tag="psd")
                        pd = pdt[0:1, :]
                        for ik in range(NT):
                            ksl = slice(ik * 128, (ik + 1) * 128)
                            ps = pss.tile([128, TC], F32, name="pss", tag="pss")
                            nc.tensor.matmul(
                                ps[:], qkT[HG + h][ik // 4][:, (ik % 4) * 128:
                                                            (ik % 4) * 128 + 128],
                                qkT[h][jq][:],
                                start=True, stop=True,
                            )
                            p = s3p.tile([128, TC], BF16, name="p", tag="p")
                            nc.scalar.activation(
                                p[:], ps[:],
                                mybir.ActivationFunctionType.Exp, scale=SCALE,
                            )
                            nc.tensor.matmul(
                                po_t[:], v_sb[ik][:, h * 128:(h + 1) * 128], p[:],
                                start=(ik == 0), stop=(ik == NT - 1),
                            )
                            nc.tensor.matmul(
                                pd, ones_sb[:, 0:1], p[:],
                                start=(ik == 0), stop=(ik == NT - 1),
                            )
                        normalize_store(jq, h, po_t, pd)

                atb0 = None
                for jq in range(NTC):
                    scope = nc.named_scope(f"attn{jq}")
                    scope.__enter__()
                    if denom == "headouter":
                        attn_chunk_headouter(jq)
                    else:
                        attn_chunk_ikouter(jq)
                    nc.gpsimd.collective_compute(
                        "AllGather",
                        mybir.AluOpType.bypass,
                        replica_groups=REPLICA_GROUPS,
                        ins=[cc_in[jq].opt()],
                        outs=[cc_out[jq].opt()],
                    )
                    if jq == 1:
                        # prefetch chunk-0 atb during attn2 (AG0 done by then)
                        atb0 = atb_load(0, nc.sync, nc.sync)
                    scope.__exit__(None, None, None)
                if debug_taps:
                    for t in range(NTC):
                        nc.scalar.dma_start(dbg_q[:, t * TC:(t + 1) * TC],
                                            qkT[0][t][:])
                        nc.scalar.dma_start(dbg_k[:, t * TC:(t + 1) * TC],
                                            qkT[HG][t][:])
                    nc.scalar.dma_start(dbg_v[:], v_sb[0][:])
                    nc.scalar.dma_start(dbg_a[:], asb[0][0][:])
                    nc.scalar.dma_start(dbg_cc[:], cc_out[0][:])

            # ---- stage 5: output projection, all chunks ----
            with (
                tc.tile_pool(name="s5o", bufs=3) as s5o,
                tc.tile_pool(name="psf", bufs=2, space="PSUM") as psf,
            ):
                for jq in range(NTC):
                    scope = nc.named_scope(f"proj{jq}")
                    scope.__enter__()
                    atb = atb0 if jq == 0 else atb_load(jq, nc.sync, nc.scalar)
                    for mql in range(TC // 128):
                        mq = jq * (TC // 128) + mql
                        pf = psf.tile([128, OC], F32, name="psf", tag="psf")
                        for k3 in range(KT):
                            nc.tensor.matmul(
                                pf[:],
                                atb[k3][:, mql * 128:(mql + 1) * 128],
                                wo_sb[k3][:],
                                start=(k3 == 0),
                                stop=(k3 == KT - 1),
                            )
                        ob = s5o.tile([128, OC], F32, name="ob", tag="ob")
                        nc.scalar.activation(
                            ob[:], pf[:], mybir.ActivationFunctionType.Copy
                        )
                        nc.sync.dma_start(out[mq * 128:(mq + 1) * 128, :], ob[:])
                    scope.__exit__(None, None, None)

    nc.compile()
    return nc


def _get_nc():
    global _NC
    if _NC is None:
        _NC = _build()
    return _NC


def _bf16(a):
    return np.ascontiguousarray(np.asarray(a).astype(ml_dtypes.bfloat16))


def _prep_in_maps(x, rope, qkv_w, out_w):
    x = np.asarray(x, np.float32)
    rope = np.asarray(rope, np.float32)
    qkv_w = np.asarray(qkv_w, np.float32)
    out_w = np.asarray(out_w, np.float32)

    freqs = rope[:, 0, :]  # [N, D]
    cosT = np.repeat(freqs[:, 0::2], 2, axis=1).T  # [D, N]
    sinT = np.repeat(freqs[:, 1::2], 2, axis=1).T.copy()
    sinT[0::2, :] *= -1.0  # rope sign folded in: rot[2i] = -q[2i+1]

    qkv3 = qkv_w.reshape(HID, 3, H, D)
    xTs = [_bf16(x[b].T) for b in range(B)]
    cosT_b, sinT_b = _bf16(cosT), _bf16(sinT)
    in_maps = []
    for core in range(8):
        b, g = core // G, core % G
        hs = slice(g * HG, (g + 1) * HG)
        wq = qkv3[:, 0, hs, :].reshape(HID, QK_COLS)
        wk = qkv3[:, 1, hs, :].reshape(HID, QK_COLS)
        in_maps.append(
            dict(
                xT=xTs[b],
                wqk=_bf16(np.concatenate([wq, wk], axis=1)),
                wv=_bf16(qkv3[:, 2, hs, :].reshape(HID, QK_COLS)),
                wo=_bf16(out_w[:, g * OC:(g + 1) * OC]),
                cosT=cosT_b,
                sinT=sinT_b,
            )
        )
    return in_maps


def kernel(x, rope, qkv_w, out_w):
    global LAST_RESULT
    nc = _get_nc()
    in_maps = _prep_in_maps(x, rope, qkv_w, out_w)
    res = run_bass_kernel_spmd(nc, in_maps, core_ids=list(range(8)))
    LAST_RESULT = res
    outs = [r["out"] for r in res.results]
    full = np.stack(
        [np.concatenate([outs[b * G + g] for g in range(G)], axis=1) for b in range(B)]
    )
    return full.astype(np.float32)
